# revision 1
# baseline (speedup 1.0000x reference)
"""Trainium2 Bass kernel for nn_Mix8Net (dense directional-conv CNN).

Data-parallel over 8 NeuronCores: batch 1024 -> 128 samples/core.

Per-core dataflow (channels on SBUF partitions, pixels*samples on free dim):
  - Activations live in a "tall" layout: per-sample block = 16 rows x 16
    cols = 256 fp32 (row 0 = zero separator, col 0 = zero border, 15x15
    image at rows 1-15 / cols 1-15).  A 3-tap directional conv tap with
    offset (dy,dx) is a matmul whose moving operand is the per-sample
    flat range [block, block+512) (2 samples) shifted by 16*dy+dx -
    out-of-image reads land on the zero separators/borders.
  - Matmuls run in float16 (1 cycle/row like bf16 but 11-bit mantissa, so
    ~6.5e-4 final rel err; FWL-able weight loads that hide behind the
    stream) accumulating fp32 in PSUM.  Moving operands are [2, 240]
    views (rows 1-15 only); outputs are matching strided PSUM views.
  - The scalar engine (ACT) is the global bottleneck (11 silu layers x 1
    elem/cycle/lane = ~1.26ms/core floor), so silu touches the 225
    interior pixels only, 8 samples (4 banks, uniform 256 stride) per
    instruction: [p, s8, 15, 15].  Borders of the trunk buffers are
    never written and stay zero from the one-time memset.  3-tap (dc)
    tiles drain in two 4-sample halves so the first half overlaps the
    remaining fills (PE never stalls on PSUM hand-back).
  - The whole kernel is emitted as one software-pipelined tile stream
    (key = 2*layer + tile, globally sorted): all 4 directions and
    adjacent chunks interleave, so heavy 3-tap fills overlap other
    layers' drains and there are no phase seams.
  - Residual adds run on VectorE over interior views; the final conv
    (COUT=64) is column-tiled (two 64-col PE tiles share each bank, 2
    samples per partition half), bias-added on VectorE straight out of
    PSUM, and DMA'd raw (host unscrambles the sample order).
  - The initial conv (CIN=2, 3 taps) is packed into one K=6 matmul per
    sample pair using host-prepared pre-shifted copies of x, placed at
    partitions 32*d (one group per direction).
"""
import numpy as np
import concourse.bacc as bacc
import concourse.mybir as mybir
import concourse.tile as tile
from concourse import bass_utils

F32 = mybir.dt.float32
BF16 = mybir.dt.float16
NPBF16 = np.float16
AF = mybir.ActivationFunctionType

DIR_OFFSETS = (
    ((0, -1), (0, 0), (0, 1)),
    ((-1, 0), (0, 0), (1, 0)),
    ((-1, -1), (0, 0), (1, 1)),
    ((1, -1), (0, 0), (-1, 1)),
)

B, CIN, H, W = 1024, 2, 15, 15
M, COUT = 128, 64
NB = 4                      # DirectionalConvResBlocks
NCORES = 8
BLOC = B // NCORES          # 128 samples per core
NS = 16                     # samples per chunk
NCH = BLOC // NS            # 8 chunks
BLK = 256                   # 16x16 per-sample block
TALLB = (NS + 2) * BLK      # chunk window incl lead/trail blocks
TOT = (BLOC + 2) * BLK      # full-core tall array length

_CACHE = {}


def _mov(buf, s0, delta=0, psl=None):
    """[p, 2, 15, 15] interior moving view of 2 sample blocks from s0."""
    a = BLK * (s0 + 1) + delta
    v = buf[:, a:a + 512] if psl is None else buf[psl, a:a + 512]
    return v.rearrange("p (s r c) -> p s r c",
                       r=16, c=16)[:, :, 1:16, 1:16]


def _mov512(buf, s0):
    """[p, 512] contiguous moving view: 2 full sample blocks from s0."""
    a = BLK * (s0 + 1)
    return buf[:, a:a + 512]


def _int8(buf, s0):
    """[p, 8, 15, 15] interior view of 8 sample blocks from sample s0."""
    a = BLK * (s0 + 1)
    v = buf[:, a:a + 8 * BLK]
    return v.rearrange("p (s r c) -> p s r c", r=16, c=16)[:, :, 1:16, 1:16]


def _ps_int(ps, np_=128):
    """[p, 8, 15, 15] interior view of a 4-bank psum tile (8 samples)."""
    v = ps[0:np_].rearrange("p b q -> p (b q)")
    return v.rearrange("p (s r c) -> p s r c", r=16, c=16)[:, :, 1:16, 1:16]


def _int4(buf, s0):
    """[p, 4, 15, 15] interior view of 4 sample blocks from sample s0."""
    a = BLK * (s0 + 1)
    v = buf[:, a:a + 4 * BLK]
    return v.rearrange("p (s r c) -> p s r c", r=16, c=16)[:, :, 1:16, 1:16]


def _ps_int4(ps, b0):
    """[p, 4, 15, 15] interior view of banks b0..b0+1 (4 samples)."""
    v = ps[:, b0:b0 + 2].rearrange("p b q -> p (b q)")
    return v.rearrange("p (s r c) -> p s r c", r=16, c=16)[:, :, 1:16, 1:16]


def _int2(buf, s0):
    """[p, 2, 15, 15] interior view of 2 sample blocks from sample s0."""
    a = BLK * (s0 + 1)
    v = buf[:, a:a + 2 * BLK]
    return v.rearrange("p (s r c) -> p s r c", r=16, c=16)[:, :, 1:16, 1:16]


def _int6(buf, s0):
    """[p, 6, 15, 15] interior view of 6 sample blocks from sample s0."""
    a = BLK * (s0 + 1)
    v = buf[:, a:a + 6 * BLK]
    return v.rearrange("p (s r c) -> p s r c", r=16, c=16)[:, :, 1:16, 1:16]


def _build(nch=NCH, af=None, trunk=None):
    af = AF.Silu if af is None else af
    TRK = BF16 if trunk is None else trunk
    nc = bacc.Bacc("TRN2", target_bir_lowering=False, debug=False)

    xt12_d = nc.dram_tensor("xt12", [4, 6, TOT], TRK, kind="ExternalInput").ap()
    wd0_d = nc.dram_tensor("wd0", [6, 128], BF16, kind="ExternalInput").ap()
    wdc_d = nc.dram_tensor("wdc", [128, 12, 128], BF16, kind="ExternalInput").ap()
    wpx_d = nc.dram_tensor("wpx", [128, NB, 128], BF16, kind="ExternalInput").ap()
    wc1_d = nc.dram_tensor("wc1", [128, 128], BF16, kind="ExternalInput").ap()
    wc2_d = nc.dram_tensor("wc2", [128, 128], BF16, kind="ExternalInput").ap()
    wf_d = nc.dram_tensor("wf", [128, 64], BF16, kind="ExternalInput").ap()
    bd0_d = nc.dram_tensor("bd0", [128, 1], F32, kind="ExternalInput").ap()
    bdc_d = nc.dram_tensor("bdc", [128, NB], F32, kind="ExternalInput").ap()
    bpx_d = nc.dram_tensor("bpx", [128, NB], F32, kind="ExternalInput").ap()
    bc1_d = nc.dram_tensor("bc1", [128, 1], F32, kind="ExternalInput").ap()
    bc2_d = nc.dram_tensor("bc2", [128, 1], F32, kind="ExternalInput").ap()
    bf2_d = nc.dram_tensor("bf2", [128, 1], F32, kind="ExternalInput").ap()
    # raw per-(chunk, dir) final output; host unscrambles the sample order
    out_d = nc.dram_tensor("out", [NCH, 4, 128, 8 * 225], F32,
                           kind="ExternalOutput")

    with tile.TileContext(nc) as tc:
        # per-direction persistent SBUF tensors (4 sets: the whole chunk
        # runs as one interleaved tile stream with no pair seams)
        xsb = [nc.alloc_sbuf_tensor(f"xs{i}", [128, TALLB], TRK).ap()
               for i in range(4)]
        t1b = [nc.alloc_sbuf_tensor(f"t1{i}", [128, TALLB], BF16).ap()
               for i in range(4)]
        t2b = [nc.alloc_sbuf_tensor(f"t2{i}", [128, TALLB], BF16).ap()
               for i in range(4)]
        xt = nc.alloc_sbuf_tensor("xt", [128, TALLB], TRK).ap()
        ofb = [nc.alloc_sbuf_tensor(f"of{i}", [128, 8 * 225], F32).ap()
               for i in range(2)]
        wd0 = nc.alloc_sbuf_tensor("wd0s", [128, 128], BF16).ap()
        wdc = nc.alloc_sbuf_tensor("wdcs", [128, 12, 128], BF16).ap()
        wpx = nc.alloc_sbuf_tensor("wpxs", [128, NB, 128], BF16).ap()
        wc1 = nc.alloc_sbuf_tensor("wc1s", [128, 128], BF16).ap()
        wc2 = nc.alloc_sbuf_tensor("wc2s", [128, 128], BF16).ap()
        wf = nc.alloc_sbuf_tensor("wfs", [128, 64], BF16).ap()
        bd0 = nc.alloc_sbuf_tensor("bd0s", [128, 1], F32).ap()
        bdc = nc.alloc_sbuf_tensor("bdcs", [128, NB], F32).ap()
        bpx = nc.alloc_sbuf_tensor("bpxs", [128, NB], F32).ap()
        bc1 = nc.alloc_sbuf_tensor("bc1s", [128, 1], F32).ap()
        bc2 = nc.alloc_sbuf_tensor("bc2s", [128, 1], F32).ap()
        bf2 = nc.alloc_sbuf_tensor("bf2s", [128, 1], F32).ap()

        with tc.tile_pool(name="psum", bufs=2, space="PSUM") as pspool:
            # first-needed loads first (init conv deps), then the rest
            for d in range(4):
                nc.sync.dma_start(wd0[32 * d:32 * d + 6, 0:128], wd0_d)
            nc.sync.dma_start(bd0[:], bd0_d)
            # zero only what the convs actually read as padding: lead and
            # trail blocks, row-0 separators and col-0 borders; t1b full
            # (px conv reads its whole 512-ranges) on the idle GPSIMD
            for xs in xsb:
                nc.vector.memset(
                    xs[:, :].rearrange("p (s q) -> p s q", q=BLK)
                    [:, 0:NS + 2:NS + 1, :], 0.0)
                nc.vector.memset(
                    xs[:, BLK:BLK + NS * BLK].rearrange(
                        "p (s q) -> p s q", q=BLK)[:, :, 0:16], 0.0)
                nc.vector.memset(
                    xs[:, BLK:BLK + NS * BLK].rearrange(
                        "p (s r c) -> p s r c",
                        r=16, c=16)[:, :, 1:16, 0:1], 0.0)
            for t1 in t1b:
                nc.gpsimd.memset(t1[:], 0.0)
            nc.sync.dma_start(wdc[:], wdc_d)
            nc.sync.dma_start(wpx[:], wpx_d)
            nc.sync.dma_start(wc1[:], wc1_d)
            nc.sync.dma_start(wc2[:], wc2_d)
            nc.sync.dma_start(wf[:], wf_d)
            for t_, d_ in [(bdc, bdc_d), (bpx, bpx_d),
                           (bc1, bc1_d), (bc2, bc2_d), (bf2, bf2_d)]:
                nc.sync.dma_start(t_[:], d_)

            def conv_tile(job, t):
                """Fill one 4-bank psum tile (8 samples) + silu drain.
                job = (src, wap, bias_ap, dst, deltas, psl, tp, res)."""
                (src, wap, bias_ap, dst, deltas, psl, tp, res) = job
                ps = pspool.tile([128, 4, 512], F32, tag="ps")
                # 3-tap fills are ~1.5x the drain time: split the drain in
                # two so the first half overlaps the remaining fills and
                # the PE never stalls on PSUM hand-back
                split = len(deltas) > 1
                for g in range(4):
                    s0 = 8 * t + 2 * g
                    pso = ps[:, g, 0:512].rearrange(
                        "p (s r c) -> p s r c",
                        r=16, c=16)[:, :, 1:16, 1:16]
                    for ti, dl in enumerate(deltas):
                        nc.tensor.matmul(
                            pso,
                            wap if len(deltas) == 1 else wap[:, ti, :],
                            _mov(src, s0, dl, psl=psl),
                            start=(ti == 0),
                            stop=(ti == len(deltas) - 1),
                            tile_position=tp,
                        )
                    if split and g % 2 == 1:
                        nc.scalar.activation(
                            _int4(dst, 8 * t + 2 * (g - 1)),
                            _ps_int4(ps, g - 1), af, bias=bias_ap)
                if not split:
                    nc.scalar.activation(_int8(dst, 8 * t), _ps_int(ps),
                                         af, bias=bias_ap)
                if res is not None:
                    nc.vector.tensor_add(_int8(res, 8 * t),
                                         _int8(res, 8 * t),
                                         _int8(dst, 8 * t))

            def final_tile(d, ch):
                """Column-tiled final conv for one direction: bank g has
                samples 4g,4g+1 on partitions 0-63 / 4g+2,4g+3 on 64-127;
                one DVE bias-drain; raw DMA (host unscrambles)."""
                xs = xsb[d]
                of = ofb[d % 2]
                ps = pspool.tile([128, 4, 512], F32, tag="ps")
                for g in range(4):
                    for h in range(2):
                        nc.tensor.matmul(
                            ps[64 * h:64 * h + 64, g, 0:512], wf[:],
                            _mov512(xs, 4 * g + 2 * h), start=True,
                            stop=True, tile_position=(0, 64 * h))
                ofv = of[:].rearrange("p (s r c) -> p s r c", r=15, c=15)
                nc.vector.tensor_scalar_add(ofv, _ps_int(ps), bf2[:])
                nc.sync.dma_start(out_d.ap()[ch, d], of[:])

            # one software-pipelined tile stream over the whole kernel:
            # layer L's tile k is keyed 2L+k, globally sorted, so heavy
            # dc fills interleave with light 1x1 fills and chunk seams
            # overlap; deps (same (d,t), next layer) stay 2 tiles ahead
            stream = []

            def add_layer(base, j, jobs):
                # layer stride 2, tile stride 1: each layer's tiles lag
                # the previous layer's by ~7 emission slots, leaving the
                # fills plenty of dependency slack
                for t in range(2):
                    for d in range(4):
                        stream.append((base + 2 * j + 4 * t + d,
                                       base // 2 + j,
                                       (conv_tile, jobs[d], t)))

            for ch in range(nch):
                base = 26 * ch
                stream.append((base - 1, base // 2, ("xt", ch)))
                add_layer(base, 0, [
                    (xt, wd0[slice(32 * d, 32 * d + 6), 0:128], bd0[:],
                     xsb[d], [0], slice(32 * d, 32 * d + 6),
                     (32 * d, 0), None)
                    for d in range(4)])
                for i in range(NB):
                    add_layer(base, 1 + 2 * i, [
                        (xsb[d], wdc[:, 3 * i:3 * i + 3, :],
                         bdc[:, i:i + 1], t1b[d],
                         [16 * dy + dx for (dy, dx) in DIR_OFFSETS[d]],
                         None, None, None)
                        for d in range(4)])
                    add_layer(base, 2 + 2 * i, [
                        (t1b[d], wpx[:, i, :], bpx[:, i:i + 1],
                         t2b[d], [0], None, None, xsb[d])
                        for d in range(4)])
                add_layer(base, 9, [
                    (xsb[d], wc1[:], bc1[:], t1b[d], [0],
                     None, None, None) for d in range(4)])
                add_layer(base, 10, [
                    (t1b[d], wc2[:], bc2[:], t2b[d], [0],
                     None, None, xsb[d]) for d in range(4)])
                for d in range(4):
                    # trails c2's (d, t1) tile (key base+24+d) so the
                    # final matmul sees both residual adds; ties with the
                    # next chunk's init resolve by layer index (final
                    # first), keeping the xs WAR order correct
                    stream.append((base + 26 + d, base // 2 + 11,
                                   ("final", d, ch)))

            stream.sort(key=lambda e: (e[0], e[1]))
            for _, _, item in stream:
                if item[0] == "xt":
                    ch = item[1]
                    a0 = BLK * NS * ch
                    for d in range(4):
                        nc.sync.dma_start(xt[32 * d:32 * d + 6, :],
                                          xt12_d[d, :, a0:a0 + TALLB])
                elif item[0] == "final":
                    final_tile(item[1], item[2])
                else:
                    fn, job, t = item
                    fn(job, t)

    nc.compile()
    return nc


def _prep(x, w_d0, b_d0, w_dc, b_dc, w_px, b_px, w_c1, b_c1, w_c2, b_c2,
          w_f, b_f, np_trunk=NPBF16):
    """Host-side packing: weights transposed to lhsT, x pre-shifted per
    direction/tap into the tall layout."""
    x = np.asarray(x, np.float32)

    # tall per-core x: [core, 2, TOT]
    xtall = np.zeros((NCORES, CIN, BLOC + 2, 16, 16), np.float32)
    xs = x.reshape(NCORES, BLOC, CIN, H, W)
    xtall[:, :, 1:BLOC + 1, 1:16, 1:16] = xs.transpose(0, 2, 1, 3, 4)
    xtall = xtall.reshape(NCORES, CIN, TOT)

    xt12 = np.zeros((NCORES, 4, 6, TOT), np.float32)
    for d in range(4):
        for t in range(3):
            dy, dx = DIR_OFFSETS[d][t]
            dl = 16 * dy + dx
            for c in range(CIN):
                srcv = xtall[:, c]
                dst = xt12[:, d, 2 * t + c]
                if dl > 0:
                    dst[:, :-dl] = srcv[:, dl:]
                elif dl < 0:
                    dst[:, -dl:] = srcv[:, :dl]
                else:
                    dst[:] = srcv

    bfc = lambda a: np.ascontiguousarray(a).astype(NPBF16)
    com = dict(
        wd0=bfc(np.asarray(w_d0, np.float32).transpose(0, 2, 1).reshape(6, 128)),
        wdc=bfc(np.asarray(w_dc, np.float32).transpose(3, 0, 1, 2).reshape(128, 12, 128)),
        wpx=bfc(np.asarray(w_px, np.float32).transpose(2, 0, 1)),
        wc1=bfc(np.asarray(w_c1, np.float32).T),
        wc2=bfc(np.asarray(w_c2, np.float32).T),
        wf=bfc(np.asarray(w_f, np.float32).T),
        bd0=np.asarray(b_d0, np.float32).reshape(128, 1),
        bdc=np.ascontiguousarray(np.asarray(b_dc, np.float32).T),
        bpx=np.ascontiguousarray(np.asarray(b_px, np.float32).T),
        bc1=np.asarray(b_c1, np.float32).reshape(128, 1),
        bc2=np.asarray(b_c2, np.float32).reshape(128, 1),
        bf2=np.ascontiguousarray(np.concatenate(
            [np.asarray(b_f, np.float32)] * 2).reshape(128, 1)),
    )
    in_maps = []
    for core in range(NCORES):
        m = dict(com)
        m["xt12"] = xt12[core].astype(np_trunk)
        in_maps.append(m)
    return in_maps


LAST_RESULT = None


def kernel(**inputs) -> np.ndarray:
    global LAST_RESULT
    if "nc" not in _CACHE:
        _CACHE["nc"] = _build()
    nc = _CACHE["nc"]
    in_maps = _prep(**inputs)
    res = bass_utils.run_bass_kernel_spmd(nc, in_maps,
                                          core_ids=list(range(NCORES)))
    LAST_RESULT = res
    # unscramble the raw column-tiled final layout:
    # raw[ch, d, (half, c), (k, 225)] -> sample 4*(k//2) + 2*half + k%2
    out = np.empty((B, 4, COUT, 225), np.float32)
    ov = out.reshape(NCORES, NCH, NS, 4, COUT, 225)
    for core, r in enumerate(res.results):
        raw = r["out"].reshape(NCH, 4, 2, COUT, 8, 225)
        for half in range(2):
            for k in range(8):
                s = 4 * (k // 2) + 2 * half + (k % 2)
                ov[core, :, s] = raw[:, :, half, :, k]
    return np.ascontiguousarray(out.reshape(B, 4, COUT, H, W))

